# Initial kernel scaffold
#
"""Sparse GQA attention (causal + sliding window + global tokens) with LoRA
projections and RoPE, distributed over 8 TRN2 NeuronCores.

Sharding: batch (2) x kv-head-group (4). Core (b, g) computes q heads
4g..4g+3 and kv head g for batch b, producing a partial output-projection
sum; the host adds the 4 group partials per batch.

Host-side preprocessing (exact, linear):
  - LoRA folded into the dense weights: W_eff = W + B @ A.
  - Weights pre-transposed into matmul (lhsT / rhs) layouts, cast to bf16.
  - q/k weight rows permuted to the rotate-half layout (evens then odds)
    so RoPE becomes half-tile multiplies (matmul accumulation in fp32).
  - x transposed to [D, S] per batch (contraction dim on partitions).
"""

import os
import sys

import numpy as np

for _p in ("/root/.axon_site", "/root/.axon_site/_ro/trn_rl_repo",
           "/root/.axon_site/_ro/pypackages", "/opt/trn_rl_repo"):
    if os.path.isdir(_p) and _p not in sys.path:
        sys.path.append(_p)

import ml_dtypes
import concourse.bacc as bacc
import concourse.mybir as mybir
import concourse.tile as tile
from concourse.bass_utils import run_bass_kernel_spmd

B, S, D = 2, 2048, 2048
H, KVH, HD = 16, 4, 128
WINDOW, GLOBAL = 512, 64
THETA = 1000000.0
NCORES = 8
GH = H // KVH          # q heads per core
GF = GH * HD           # 512 projection features per core
TOK = 512              # token chunk for projections
NCH = S // TOK
NQB = S // 128         # 16 query blocks
NKB = S // 128
WBL = WINDOW // 128    # 4 window blocks before the diagonal
INV_SQRT = 1.0 / float(np.sqrt(HD))
NEG = -1e9

F32 = mybir.dt.float32
BF16 = mybir.dt.bfloat16

_PROGRAM = {}


def _emit(nc, t):
    """Emit the per-core Tile program. `t` maps input names to DRAM APs."""
    tc = t["tc"]
    from contextlib import ExitStack

    xTd = t["xt"].ap()      # [NCH,128,16,TOK] chunk-packed
    wqT = t["wqt"].ap()     # [128,16,GF]
    wkT = t["wkt"].ap()     # [128,16,HD]
    wvT = t["wvt"].ap()
    woT = t["wot"].ap()     # [128,GH,D]
    y = t["y"].ap()         # [S, D]

    with ExitStack() as stk:
        singles = stk.enter_context(tc.tile_pool(name="singles", bufs=1))
        persist = stk.enter_context(tc.tile_pool(name="persist", bufs=1))

        ident_sb = singles.tile([128, 128], BF16)
        nc.sync.dma_start(out=ident_sb, in_=t["ident"].ap())
        tri640_sb = singles.tile([128, 640], BF16)
        nc.sync.dma_start(out=tri640_sb, in_=t["tri640"].ap())
        edge_sb = singles.tile([128, 128], BF16)
        nc.sync.dma_start(out=edge_sb, in_=t["edge"].ap())
        edgeg_sb = singles.tile([128, 128], BF16)
        nc.sync.dma_start(out=edgeg_sb, in_=t["edgeg"].ap())

        qT_sb = persist.tile([128, GH, S], BF16)    # [hd, head, tok]
        kT_sb = persist.tile([128, S], BF16)        # [hd, tok]
        v_sb = persist.tile([128, NKB, HD], BF16)   # [tok%128, kblock, hd]
        oT_sb = persist.tile([128, GH, S], BF16)    # [hd, head, tok]

        # ---------------- Phase A: projections + RoPE ----------------
        with ExitStack() as stka:
            wpool = stka.enter_context(tc.tile_pool(name="wpool", bufs=1))
            xpool = stka.enter_context(tc.tile_pool(name="xpool", bufs=2))
            tmp = stka.enter_context(tc.tile_pool(name="tmpa", bufs=2))
            psa = stka.enter_context(tc.tile_pool(name="psa", bufs=2,
                                                  space="PSUM"))

            xt0 = xpool.tile([128, 16, TOK], BF16, tag="xt")
            nc.sync.dma_start(out=xt0, in_=xTd[0])
            wq_sb = wpool.tile([128, 16, GF], BF16)
            nc.sync.dma_start(out=wq_sb, in_=wqT)
            wk_sb = wpool.tile([128, 16, HD], BF16)
            nc.sync.dma_start(out=wk_sb, in_=wkT)
            wv_sb = wpool.tile([128, 16, HD], BF16)
            nc.sync.dma_start(out=wv_sb, in_=wvT)
            cos_sb = wpool.tile([128, S], F32)
            nc.sync.dma_start(out=cos_sb, in_=t["cos2t"].ap())
            sin_sb = wpool.tile([128, S], F32)
            nc.sync.dma_start(out=sin_sb, in_=t["sins2t"].ap())

            for c in range(NCH):
                cs = slice(c * TOK, (c + 1) * TOK)
                if c == 0:
                    xt = xt0
                else:
                    xt = xpool.tile([128, 16, TOK], BF16, tag="xt")
                    nc.sync.dma_start(out=xt, in_=xTd[c])

                # q (GH heads) and k (1 head) with rotate-half RoPE
                for h in range(GH + 1):
                    if h < GH:
                        wslc = wq_sb[:, :, h * HD:(h + 1) * HD]
                        dst = qT_sb[:, h, cs]
                    else:
                        wslc = wk_sb
                        dst = kT_sb[:, cs]
                    pq = psa.tile([128, TOK], F32, tag="pq")
                    for a in range(16):
                        nc.tensor.matmul(pq, wslc[:, a, :], xt[:, a, :],
                                         start=(a == 0), stop=(a == 15))
                    # t1 = rotate_half(pq) * sin_signed   (cross-partition)
                    t1 = tmp.tile([128, TOK], F32, tag="t1")
                    nc.vector.tensor_mul(t1[0:64, :], pq[64:128, :],
                                         sin_sb[0:64, cs])
                    nc.vector.tensor_mul(t1[64:128, :], pq[0:64, :],
                                         sin_sb[64:128, cs])
                    t2 = tmp.tile([128, TOK], F32, tag="t2")
                    nc.vector.tensor_mul(t2, pq, cos_sb[:, cs])
                    nc.vector.tensor_add(dst, t2, t1)

                # v: compute vT then transpose to natural [tok, hd] bf16
                pv = psa.tile([128, TOK], F32, tag="pq")
                for a in range(16):
                    nc.tensor.matmul(pv, wv_sb[:, a, :], xt[:, a, :],
                                     start=(a == 0), stop=(a == 15))
                vt = tmp.tile([128, TOK], BF16, tag="vt")
                nc.scalar.copy(vt, pv)
                for b2 in range(TOK // 128):
                    vtp = psa.tile([128, 128], BF16, tag="vtp")
                    nc.tensor.transpose(vtp, vt[:, b2 * 128:(b2 + 1) * 128],
                                        ident_sb)
                    kb = c * (TOK // 128) + b2
                    nc.vector.tensor_copy(v_sb[:, kb, :], vtp)

        # woT load overlaps attention (pool entered after phase-A release)
        wopool = tc.tile_pool(name="wopool", bufs=1)
        wopool_cm = wopool.__enter__()
        wo_sb = wopool_cm.tile([128, GH, D], BF16)
        nc.sync.dma_start(out=wo_sb, in_=woT)

        # ------- Phase B + C: sparse attention, interleaved projection -------
        with ExitStack() as stkb:
            bpool = stkb.enter_context(tc.tile_pool(name="bpool", bufs=2))
            spool = stkb.enter_context(tc.tile_pool(name="spool", bufs=4))
            cpool = stkb.enter_context(tc.tile_pool(name="cpool", bufs=2))
            psb = stkb.enter_context(tc.tile_pool(name="psb", bufs=1,
                                                  space="PSUM"))

            for qi in range(NQB):
                for h in range(GH):
                    qlhs = qT_sb[:, h, qi * 128:(qi + 1) * 128]
                    ps = psb.tile([128, 704], F32, tag="scores", bufs=2)
                    if qi <= 3:
                        W = (qi + 1) * 128
                        nc.tensor.matmul(ps[:, 0:W], ident_sb,
                                         tri640_sb[:, 512 - qi * 128:
                                                   512 + 128],
                                         start=True, stop=True)
                        nc.tensor.matmul(ps[:, 0:W], qlhs, kT_sb[:, 0:W],
                                         start=False, stop=True)
                        nblk = qi + 1
                    else:
                        W = 640 if qi == 4 else 704
                        k0 = (qi - WBL) * 128
                        nc.tensor.matmul(ps[:, 0:128], ident_sb,
                                         edgeg_sb if qi == 4 else edge_sb,
                                         start=True, stop=True)
                        nc.tensor.matmul(ps[:, 0:128], qlhs,
                                         kT_sb[:, k0:k0 + 128],
                                         start=False, stop=True)
                        nc.tensor.matmul(ps[:, 128:512], qlhs,
                                         kT_sb[:, k0 + 128:k0 + 512],
                                         start=True, stop=True)
                        nc.tensor.matmul(ps[:, 512:640], ident_sb,
                                         tri640_sb[:, 512:640],
                                         start=True, stop=True)
                        nc.tensor.matmul(ps[:, 512:640], qlhs,
                                         kT_sb[:, qi * 128:(qi + 1) * 128],
                                         start=False, stop=True)
                        if qi > 4:
                            nc.tensor.matmul(ps[:, 640:704], qlhs,
                                             kT_sb[:, 0:64],
                                             start=True, stop=True)
                        nblk = 5

                    sums = spool.tile([128, 1], F32, tag="sums")
                    p_sb = bpool.tile([128, 704], BF16, tag="p")
                    nc.scalar.activation(p_sb[:, 0:W], ps[:, 0:W],
                                         mybir.ActivationFunctionType.Exp,
                                         scale=INV_SQRT, accum_out=sums)
                    inv = spool.tile([128, 1], F32, tag="inv")
                    nc.vector.reciprocal(inv, sums)
                    diag = spool.tile([128, 128], BF16, tag="diag")
                    nc.vector.tensor_scalar_mul(diag, ident_sb, inv)

                    # normalized transpose via matmul with rhs=diag(inv)
                    ptp = psb.tile([128, 768], F32, tag="ptp", bufs=1)
                    for i in range(nblk):
                        nc.tensor.matmul(ptp[:, i * 128:(i + 1) * 128],
                                         p_sb[:, i * 128:(i + 1) * 128],
                                         diag, start=True, stop=True)
                    if qi > 4:
                        nc.tensor.matmul(ptp[0:64, 640:768],
                                         p_sb[:, 640:704], diag,
                                         start=True, stop=True)
                    cw = 768 if qi > 4 else nblk * 128
                    pt = bpool.tile([128, 768], BF16, tag="pt", bufs=2)
                    nc.vector.tensor_copy(pt[:, 0:cw], ptp[:, 0:cw])

                    po = psb.tile([128, 128], F32, tag="po", bufs=1)
                    kbs = (list(range(qi + 1)) if qi <= 4 else
                           list(range(qi - WBL, qi + 1)))
                    n_mm = len(kbs) + (1 if qi > 4 else 0)
                    for i, kb in enumerate(kbs):
                        nc.tensor.matmul(po, v_sb[:, kb, :],
                                         pt[:, i * 128:(i + 1) * 128],
                                         start=(i == 0),
                                         stop=(i == n_mm - 1))
                    if qi > 4:
                        nc.tensor.matmul(po, v_sb[0:64, 0, :],
                                         pt[0:64, 640:768],
                                         start=False, stop=True)
                    nc.scalar.copy(oT_sb[:, h, qi * 128:(qi + 1) * 128], po)

                # output projection for this token block
                ts = slice(qi * 128, (qi + 1) * 128)
                ysb = cpool.tile([128, D], BF16, tag="ysb")
                for cchunk in range(4):
                    ns = slice(cchunk * 512, (cchunk + 1) * 512)
                    py = psb.tile([128, 512], F32, tag="py", bufs=1)
                    for hh in range(GH):
                        nc.tensor.matmul(py, oT_sb[:, hh, ts],
                                         wo_sb[:, hh, ns],
                                         start=(hh == 0), stop=(hh == GH - 1))
                    nc.vector.tensor_copy(ysb[:, ns], py)
                nc.sync.dma_start(out=y[ts, :], in_=ysb)

        wopool.__exit__(None, None, None)


def _build_program():
    if "nc" in _PROGRAM:
        return _PROGRAM["nc"]
    nc = bacc.Bacc("TRN2", target_bir_lowering=False, debug=False,
                   num_devices=NCORES)
    t = {}
    t["xt"] = nc.dram_tensor("xt", [NCH, 128, 16, TOK], BF16,
                             kind="ExternalInput")
    t["wqt"] = nc.dram_tensor("wqt", [128, 16, GF], BF16,
                              kind="ExternalInput")
    t["wkt"] = nc.dram_tensor("wkt", [128, 16, HD], BF16,
                              kind="ExternalInput")
    t["wvt"] = nc.dram_tensor("wvt", [128, 16, HD], BF16,
                              kind="ExternalInput")
    t["wot"] = nc.dram_tensor("wot", [128, GH, D], BF16,
                              kind="ExternalInput")
    t["cos2t"] = nc.dram_tensor("cos2t", [128, S], F32, kind="ExternalInput")
    t["sins2t"] = nc.dram_tensor("sins2t", [128, S], F32,
                                 kind="ExternalInput")
    t["ident"] = nc.dram_tensor("ident", [128, 128], BF16,
                                kind="ExternalInput")
    t["tri640"] = nc.dram_tensor("tri640", [128, 640], BF16,
                                 kind="ExternalInput")
    t["edge"] = nc.dram_tensor("edge", [128, 128], BF16, kind="ExternalInput")
    t["edgeg"] = nc.dram_tensor("edgeg", [128, 128], BF16,
                                kind="ExternalInput")
    t["y"] = nc.dram_tensor("y", [S, D], BF16, kind="ExternalOutput")

    with tile.TileContext(nc) as tc:
        t["tc"] = tc
        _emit(nc, t)
    nc.compile()
    _PROGRAM["nc"] = nc
    return nc


def _host_inputs(x, wq_w, wq_a, wq_b, wk_w, wk_a, wk_b, wv_w, wv_a, wv_b,
                 wo_w, wo_a, wo_b):
    f32 = np.float32
    bf16 = ml_dtypes.bfloat16
    Wq = (wq_w.astype(f32) + wq_b.astype(f32) @ wq_a.astype(f32))
    Wk = (wk_w.astype(f32) + wk_b.astype(f32) @ wk_a.astype(f32))
    Wv = (wv_w.astype(f32) + wv_b.astype(f32) @ wv_a.astype(f32))
    Wo = (wo_w.astype(f32) + wo_b.astype(f32) @ wo_a.astype(f32))

    perm = np.concatenate([np.arange(0, HD, 2), np.arange(1, HD, 2)])
    Wq_p = Wq.reshape(H, HD, D)[:, perm, :].reshape(H * HD, D)
    Wk_p = Wk.reshape(KVH, HD, D)[:, perm, :].reshape(KVH * HD, D)

    j = np.arange(HD // 2, dtype=np.float64)
    inv_freq = 1.0 / THETA ** (2.0 * j / HD)
    tpos = np.arange(S, dtype=np.float64)
    freqs = np.outer(inv_freq, tpos)                      # [64, S]
    cosT = np.cos(freqs)
    sinT = np.sin(freqs)
    cos2t = np.concatenate([cosT, cosT], 0).astype(f32)
    sins2t = np.concatenate([-sinT, sinT], 0).astype(f32)

    a = np.arange(128)
    tri = np.where(a[:, None] >= a[None, :], 0.0, NEG)
    tri640 = np.concatenate([np.zeros((128, 512)), tri], 1).astype(bf16)
    edge = np.where(a[None, :] > a[:, None], 0.0, NEG).astype(bf16)
    edgeg = np.where((a[None, :] > a[:, None]) | (a[None, :] < GLOBAL),
                     0.0, NEG).astype(bf16)
    ident = np.eye(128, dtype=bf16)

    common = dict(cos2t=cos2t, sins2t=sins2t, tri640=tri640, edge=edge,
                  edgeg=edgeg, ident=ident)
    def pack_w(wT, nf):
        # [D, nf] -> [128, 16, nf], partition-contiguous
        return np.ascontiguousarray(
            wT.reshape(16, 128, nf).transpose(1, 0, 2)).astype(bf16)

    NCH_ = S // TOK
    in_maps = []
    for b in range(B):
        xT = x[b].astype(f32).T.astype(bf16)            # [D, S]
        xh = np.ascontiguousarray(
            xT.reshape(16, 128, NCH_, TOK).transpose(2, 1, 0, 3))
        for g in range(KVH):
            woT = Wo[:, GF * g:GF * (g + 1)].T          # [GF, D]
            woh = np.ascontiguousarray(
                woT.reshape(GH, 128, D).transpose(1, 0, 2)).astype(bf16)
            in_maps.append(dict(
                xt=xh,
                wqt=pack_w(Wq_p[GF * g:GF * (g + 1), :].T, GF),
                wkt=pack_w(Wk_p[HD * g:HD * (g + 1), :].T, HD),
                wvt=pack_w(Wv[HD * g:HD * (g + 1), :].T, HD),
                wot=woh,
                **common,
            ))
    return in_maps


def kernel(**inputs):
    nc = _build_program()
    in_maps = _host_inputs(**inputs)
    res = None
    last_err = None
    for _attempt in range(3):
        try:
            res = run_bass_kernel_spmd(nc, in_maps,
                                       core_ids=list(range(NCORES)))
            break
        except Exception as e:  # transient first-exec device hiccups
            last_err = e
            import time as _time
            _time.sleep(2.0)
    if res is None:
        raise last_err
    out = np.zeros((B, S, D), dtype=np.float32)
    for b in range(B):
        for g in range(KVH):
            out[b] += res.results[b * KVH + g]["y"].astype(np.float32)
    return out



# revision 1
# speedup vs baseline: 1.1689x; 1.1689x over previous
"""Sparse GQA attention (causal + sliding window + global tokens) with LoRA
projections and RoPE, distributed over 8 TRN2 NeuronCores.

Sharding: batch (2) x kv-head-group (4). Core (b, g) computes q heads
4g..4g+3 and kv head g for batch b, producing a partial output-projection
sum; the host adds the 4 group partials per batch.

Host-side preprocessing (exact, linear):
  - LoRA folded into the dense weights: W_eff = W + B @ A.
  - Weights pre-transposed into matmul (lhsT / rhs) layouts, cast to bf16.
  - q/k weight rows permuted to the rotate-half layout (evens then odds)
    so RoPE becomes half-tile multiplies (matmul accumulation in fp32).
  - x transposed to [D, S] per batch (contraction dim on partitions).
"""

import os
import sys

import numpy as np

for _p in ("/root/.axon_site", "/root/.axon_site/_ro/trn_rl_repo",
           "/root/.axon_site/_ro/pypackages", "/opt/trn_rl_repo"):
    if os.path.isdir(_p) and _p not in sys.path:
        sys.path.append(_p)

import ml_dtypes
import concourse.bacc as bacc
import concourse.mybir as mybir
import concourse.tile as tile
from concourse.bass_utils import run_bass_kernel_spmd

B, S, D = 2, 2048, 2048
H, KVH, HD = 16, 4, 128
WINDOW, GLOBAL = 512, 64
THETA = 1000000.0
NCORES = 8
GH = H // KVH          # q heads per core
GF = GH * HD           # 512 projection features per core
TOK = 512              # token chunk for projections
NCH = S // TOK
NQB = S // 128         # 16 query blocks
NKB = S // 128
WBL = WINDOW // 128    # 4 window blocks before the diagonal
INV_SQRT = 1.0 / float(np.sqrt(HD))
NEG = -1e9

F32 = mybir.dt.float32
BF16 = mybir.dt.bfloat16

_PROGRAM = {}


def _emit(nc, t):
    """Emit the per-core Tile program. `t` maps input names to DRAM APs."""
    tc = t["tc"]
    from contextlib import ExitStack

    xTd = t["xt"].ap()      # [NCH,128,16,TOK] chunk-packed
    wqT = t["wqt"].ap()     # [128,16,GF]
    wkT = t["wkt"].ap()     # [128,16,HD]
    wvT = t["wvt"].ap()
    woT = t["wot"].ap()     # [128,GH,D]
    y = t["y"].ap()         # [S, D]

    with ExitStack() as stk:
        singles = stk.enter_context(tc.tile_pool(name="singles", bufs=1))
        persist = stk.enter_context(tc.tile_pool(name="persist", bufs=1))

        ident_sb = singles.tile([128, 128], BF16)
        nc.sync.dma_start(out=ident_sb, in_=t["ident"].ap())
        tri640_sb = singles.tile([128, 640], BF16)
        nc.sync.dma_start(out=tri640_sb, in_=t["tri640"].ap())
        edge_sb = singles.tile([128, 128], BF16)
        nc.sync.dma_start(out=edge_sb, in_=t["edge"].ap())
        edgeg_sb = singles.tile([128, 128], BF16)
        nc.sync.dma_start(out=edgeg_sb, in_=t["edgeg"].ap())

        qT_sb = persist.tile([128, GH, S], BF16)    # [hd, head, tok]
        kT_sb = persist.tile([128, S], BF16)        # [hd, tok]
        v_sb = persist.tile([128, NKB, HD], BF16)   # [tok%128, kblock, hd]
        oT_sb = persist.tile([128, GH, S], BF16)    # [hd, head, tok]

        # ---------------- Phase A: projections + RoPE ----------------
        with ExitStack() as stka:
            wpool = stka.enter_context(tc.tile_pool(name="wpool", bufs=1))
            xpool = stka.enter_context(tc.tile_pool(name="xpool", bufs=2))
            tmp = stka.enter_context(tc.tile_pool(name="tmpa", bufs=2))
            psa = stka.enter_context(tc.tile_pool(name="psa", bufs=2,
                                                  space="PSUM"))

            xt0 = xpool.tile([128, 16, TOK], BF16, tag="xt")
            nc.sync.dma_start(out=xt0, in_=xTd[0])
            wq_sb = wpool.tile([128, 16, GF], BF16)
            nc.sync.dma_start(out=wq_sb, in_=wqT)
            wk_sb = wpool.tile([128, 16, HD], BF16)
            nc.sync.dma_start(out=wk_sb, in_=wkT)
            wv_sb = wpool.tile([128, 16, HD], BF16)
            nc.sync.dma_start(out=wv_sb, in_=wvT)
            cos_sb = wpool.tile([128, S], F32)
            nc.sync.dma_start(out=cos_sb, in_=t["cos2t"].ap())
            sin_sb = wpool.tile([128, S], F32)
            nc.sync.dma_start(out=sin_sb, in_=t["sins2t"].ap())

            for c in range(NCH):
                cs = slice(c * TOK, (c + 1) * TOK)
                if c == 0:
                    xt = xt0
                else:
                    xt = xpool.tile([128, 16, TOK], BF16, tag="xt")
                    nc.sync.dma_start(out=xt, in_=xTd[c])

                # q (GH heads) and k (1 head) with rotate-half RoPE
                for h in range(GH + 1):
                    if h < GH:
                        wslc = wq_sb[:, :, h * HD:(h + 1) * HD]
                        dst = qT_sb[:, h, cs]
                    else:
                        wslc = wk_sb
                        dst = kT_sb[:, cs]
                    pq = psa.tile([128, TOK], F32, tag="pq")
                    for a in range(16):
                        nc.tensor.matmul(pq, wslc[:, a, :], xt[:, a, :],
                                         start=(a == 0), stop=(a == 15))
                    # t1 = rotate_half(pq) * sin_signed   (cross-partition)
                    t1 = tmp.tile([128, TOK], F32, tag="t1")
                    nc.vector.tensor_mul(t1[0:64, :], pq[64:128, :],
                                         sin_sb[0:64, cs])
                    nc.vector.tensor_mul(t1[64:128, :], pq[0:64, :],
                                         sin_sb[64:128, cs])
                    t2 = tmp.tile([128, TOK], F32, tag="t2")
                    nc.vector.tensor_mul(t2, pq, cos_sb[:, cs])
                    nc.vector.tensor_add(dst, t2, t1)

                # v: compute vT then transpose to natural [tok, hd] bf16
                pv = psa.tile([128, TOK], F32, tag="pq")
                for a in range(16):
                    nc.tensor.matmul(pv, wv_sb[:, a, :], xt[:, a, :],
                                     start=(a == 0), stop=(a == 15))
                vt = tmp.tile([128, TOK], BF16, tag="vt")
                nc.scalar.copy(vt, pv)
                for b2 in range(TOK // 128):
                    vtp = psa.tile([128, 128], BF16, tag="vtp")
                    nc.tensor.transpose(vtp, vt[:, b2 * 128:(b2 + 1) * 128],
                                        ident_sb)
                    kb = c * (TOK // 128) + b2
                    nc.vector.tensor_copy(v_sb[:, kb, :], vtp)

        # woT load overlaps attention (pool entered after phase-A release)
        wopool = tc.tile_pool(name="wopool", bufs=1)
        wopool_cm = wopool.__enter__()
        wo_sb = wopool_cm.tile([128, GH, D], BF16)
        nc.sync.dma_start(out=wo_sb, in_=woT)

        # ------- Phase B + C: sparse attention, interleaved projection -------
        with ExitStack() as stkb:
            bpool = stkb.enter_context(tc.tile_pool(name="bpool", bufs=2))
            spool = stkb.enter_context(tc.tile_pool(name="spool", bufs=4))
            cpool = stkb.enter_context(tc.tile_pool(name="cpool", bufs=2))
            psb = stkb.enter_context(tc.tile_pool(name="psb", bufs=1,
                                                  space="PSUM"))

            for qi in range(NQB):
                for h in range(GH):
                    qlhs = qT_sb[:, h, qi * 128:(qi + 1) * 128]
                    ps = psb.tile([128, 704], F32, tag="scores", bufs=2)
                    if qi <= 3:
                        W = (qi + 1) * 128
                        nc.tensor.matmul(ps[:, 0:W], ident_sb,
                                         tri640_sb[:, 512 - qi * 128:
                                                   512 + 128],
                                         start=True, stop=True)
                        nc.tensor.matmul(ps[:, 0:W], qlhs, kT_sb[:, 0:W],
                                         start=False, stop=True)
                        nblk = qi + 1
                    else:
                        W = 640 if qi == 4 else 704
                        k0 = (qi - WBL) * 128
                        nc.tensor.matmul(ps[:, 0:128], ident_sb,
                                         edgeg_sb if qi == 4 else edge_sb,
                                         start=True, stop=True)
                        nc.tensor.matmul(ps[:, 0:128], qlhs,
                                         kT_sb[:, k0:k0 + 128],
                                         start=False, stop=True)
                        nc.tensor.matmul(ps[:, 128:512], qlhs,
                                         kT_sb[:, k0 + 128:k0 + 512],
                                         start=True, stop=True)
                        nc.tensor.matmul(ps[:, 512:640], ident_sb,
                                         tri640_sb[:, 512:640],
                                         start=True, stop=True)
                        nc.tensor.matmul(ps[:, 512:640], qlhs,
                                         kT_sb[:, qi * 128:(qi + 1) * 128],
                                         start=False, stop=True)
                        if qi > 4:
                            nc.tensor.matmul(ps[:, 640:704], qlhs,
                                             kT_sb[:, 0:64],
                                             start=True, stop=True)
                        nblk = 5

                    sums = spool.tile([128, 1], F32, tag="sums")
                    p_sb = bpool.tile([128, 704], BF16, tag="p")
                    nc.scalar.activation(p_sb[:, 0:W], ps[:, 0:W],
                                         mybir.ActivationFunctionType.Exp,
                                         scale=INV_SQRT, accum_out=sums)
                    inv = spool.tile([128, 1], F32, tag="inv")
                    nc.vector.reciprocal(inv, sums)
                    diag = spool.tile([128, 128], BF16, tag="diag")
                    nc.vector.tensor_scalar_mul(diag, ident_sb, inv)

                    # normalized transpose via matmul with rhs=diag(inv)
                    ptp = psb.tile([128, 768], F32, tag="ptp", bufs=1)
                    for i in range(nblk):
                        nc.tensor.matmul(ptp[:, i * 128:(i + 1) * 128],
                                         p_sb[:, i * 128:(i + 1) * 128],
                                         diag, start=True, stop=True)
                    if qi > 4:
                        nc.tensor.matmul(ptp[0:64, 640:768],
                                         p_sb[:, 640:704], diag,
                                         start=True, stop=True)
                    cw = 768 if qi > 4 else nblk * 128
                    pt = bpool.tile([128, 768], BF16, tag="pt", bufs=2)
                    nc.vector.tensor_copy(pt[:, 0:cw], ptp[:, 0:cw])

                    po = psb.tile([128, 128], F32, tag="po", bufs=1)
                    kbs = (list(range(qi + 1)) if qi <= 4 else
                           list(range(qi - WBL, qi + 1)))
                    n_mm = len(kbs) + (1 if qi > 4 else 0)
                    for i, kb in enumerate(kbs):
                        nc.tensor.matmul(po, v_sb[:, kb, :],
                                         pt[:, i * 128:(i + 1) * 128],
                                         start=(i == 0),
                                         stop=(i == n_mm - 1))
                    if qi > 4:
                        nc.tensor.matmul(po, v_sb[0:64, 0, :],
                                         pt[0:64, 640:768],
                                         start=False, stop=True)
                    nc.scalar.copy(oT_sb[:, h, qi * 128:(qi + 1) * 128], po)

                # output projection for this token block
                ts = slice(qi * 128, (qi + 1) * 128)
                ysb = cpool.tile([128, D], BF16, tag="ysb")
                for cchunk in range(4):
                    ns = slice(cchunk * 512, (cchunk + 1) * 512)
                    py = psb.tile([128, 512], F32, tag="py", bufs=1)
                    for hh in range(GH):
                        nc.tensor.matmul(py, oT_sb[:, hh, ts],
                                         wo_sb[:, hh, ns],
                                         start=(hh == 0), stop=(hh == GH - 1))
                    nc.vector.tensor_copy(ysb[:, ns], py)
                nc.sync.dma_start(out=y[ts, :], in_=ysb)

        wopool.__exit__(None, None, None)


def _build_program():
    if "nc" in _PROGRAM:
        return _PROGRAM["nc"]
    nc = bacc.Bacc("TRN2", target_bir_lowering=False, debug=False,
                   num_devices=NCORES)
    t = {}
    t["xt"] = nc.dram_tensor("xt", [NCH, 128, 16, TOK], BF16,
                             kind="ExternalInput")
    t["wqt"] = nc.dram_tensor("wqt", [128, 16, GF], BF16,
                              kind="ExternalInput")
    t["wkt"] = nc.dram_tensor("wkt", [128, 16, HD], BF16,
                              kind="ExternalInput")
    t["wvt"] = nc.dram_tensor("wvt", [128, 16, HD], BF16,
                              kind="ExternalInput")
    t["wot"] = nc.dram_tensor("wot", [128, GH, D], BF16,
                              kind="ExternalInput")
    t["cos2t"] = nc.dram_tensor("cos2t", [128, S], F32, kind="ExternalInput")
    t["sins2t"] = nc.dram_tensor("sins2t", [128, S], F32,
                                 kind="ExternalInput")
    t["ident"] = nc.dram_tensor("ident", [128, 128], BF16,
                                kind="ExternalInput")
    t["tri640"] = nc.dram_tensor("tri640", [128, 640], BF16,
                                 kind="ExternalInput")
    t["edge"] = nc.dram_tensor("edge", [128, 128], BF16, kind="ExternalInput")
    t["edgeg"] = nc.dram_tensor("edgeg", [128, 128], BF16,
                                kind="ExternalInput")
    t["y"] = nc.dram_tensor("y", [S, D], BF16, kind="ExternalOutput")

    with tile.TileContext(nc) as tc:
        t["tc"] = tc
        _emit(nc, t)
    nc.compile()
    _PROGRAM["nc"] = nc
    return nc


def _host_inputs(x, wq_w, wq_a, wq_b, wk_w, wk_a, wk_b, wv_w, wv_a, wv_b,
                 wo_w, wo_a, wo_b):
    f32 = np.float32
    bf16 = ml_dtypes.bfloat16
    Wq = (wq_w.astype(f32) + wq_b.astype(f32) @ wq_a.astype(f32))
    Wk = (wk_w.astype(f32) + wk_b.astype(f32) @ wk_a.astype(f32))
    Wv = (wv_w.astype(f32) + wv_b.astype(f32) @ wv_a.astype(f32))
    Wo = (wo_w.astype(f32) + wo_b.astype(f32) @ wo_a.astype(f32))

    perm = np.concatenate([np.arange(0, HD, 2), np.arange(1, HD, 2)])
    Wq_p = Wq.reshape(H, HD, D)[:, perm, :].reshape(H * HD, D)
    Wk_p = Wk.reshape(KVH, HD, D)[:, perm, :].reshape(KVH * HD, D)

    j = np.arange(HD // 2, dtype=np.float64)
    inv_freq = 1.0 / THETA ** (2.0 * j / HD)
    tpos = np.arange(S, dtype=np.float64)
    freqs = np.outer(inv_freq, tpos)                      # [64, S]
    cosT = np.cos(freqs)
    sinT = np.sin(freqs)
    cos2t = np.concatenate([cosT, cosT], 0).astype(f32)
    sins2t = np.concatenate([-sinT, sinT], 0).astype(f32)

    a = np.arange(128)
    tri = np.where(a[:, None] >= a[None, :], 0.0, NEG)
    tri640 = np.concatenate([np.zeros((128, 512)), tri], 1).astype(bf16)
    edge = np.where(a[None, :] > a[:, None], 0.0, NEG).astype(bf16)
    edgeg = np.where((a[None, :] > a[:, None]) | (a[None, :] < GLOBAL),
                     0.0, NEG).astype(bf16)
    ident = np.eye(128, dtype=bf16)

    common = dict(cos2t=cos2t, sins2t=sins2t, tri640=tri640, edge=edge,
                  edgeg=edgeg, ident=ident)
    def pack_w(wT, nf):
        # [D, nf] -> [128, 16, nf], partition-contiguous
        return np.ascontiguousarray(
            wT.reshape(16, 128, nf).transpose(1, 0, 2)).astype(bf16)

    NCH_ = S // TOK
    in_maps = []
    for b in range(B):
        xT = x[b].astype(f32).T.astype(bf16)            # [D, S]
        xh = np.ascontiguousarray(
            xT.reshape(16, 128, NCH_, TOK).transpose(2, 1, 0, 3))
        for g in range(KVH):
            woT = Wo[:, GF * g:GF * (g + 1)].T          # [GF, D]
            woh = np.ascontiguousarray(
                woT.reshape(GH, 128, D).transpose(1, 0, 2)).astype(bf16)
            in_maps.append(dict(
                xt=xh,
                wqt=pack_w(Wq_p[GF * g:GF * (g + 1), :].T, GF),
                wkt=pack_w(Wk_p[HD * g:HD * (g + 1), :].T, HD),
                wvt=pack_w(Wv[HD * g:HD * (g + 1), :].T, HD),
                wot=woh,
                **common,
            ))
    return in_maps


def kernel(**inputs):
    nc = _build_program()
    in_maps = _host_inputs(**inputs)
    res = None
    last_err = None
    for _attempt in range(3):
        try:
            res = run_bass_kernel_spmd(nc, in_maps,
                                       core_ids=list(range(NCORES)))
            break
        except Exception as e:  # transient first-exec device hiccups
            last_err = e
            import time as _time
            _time.sleep(2.0)
    if res is None:
        raise last_err
    out = np.zeros((B, S, D), dtype=np.float32)
    for b in range(B):
        for g in range(KVH):
            out[b] += res.results[b * KVH + g]["y"].astype(np.float32)
    return out

